# revision 73
# baseline (speedup 1.0000x reference)
"""Trainium2 Bass kernel for MultiHeadLinearAttention.

Problem: B=4, S=2048, D=1024, H=16 heads of hd=64.
  q,k,v = x@W + b ; q,k = elu(q|k)+1
  recurrent scan: s += k_t v_t^T ; z += k_t ; out_t = (q_t s)/(q_t z + 1e-6)
  y = out @ Wo + bo

Strategy (8 NeuronCores):
  core c -> batch b = c//2, heads hh = (c%2)*8 .. +8  (column-sliced Wq/Wk/Wv,
  row-sliced Wo; the two cores of a batch produce partial output-projection
  sums that the host adds together, plus bo).

  Linear attention is computed in chunked (block-parallel) form with chunk
  C=128: per chunk and head,
    AT[j,i] = sum_d k[j,d] q[i,d]          (j,i in chunk; masked to j<=i)
    acc[i,0:64]  = sum_j AT_m[j,i] v[j,:] + q_i @ S_pre
    acc[i,64]    = sum_j AT_m[j,i] * 1    + q_i @ z_pre    ([V|1] interleave)
    out_i = acc[i,0:64] / (acc[i,64] + 1e-6)
    [S|z] += K_c^T [V_c|1]
  elu(x)+1 == min(exp(x),1) + max(x,0).

  The host feeds x[b].T so projections contract along the partition dim and
  produce Q^T,K^T,V^T directly; K/V natural layouts come from PE transposes.
"""

import os
import sys

import numpy as np
import ml_dtypes

sys.path.insert(0, "/opt/trn_rl_repo")

B, S, D = 4, 2048, 1024
H, HD = 16, 64
HPC = 8           # heads per core
MC = HPC * HD     # 512 head-dims per core
C = 128           # attention chunk
SB = 512          # s-block
NBLK = S // SB    # 4
NST = SB // C     # s-tiles (=chunks) per block

# matmul operand dtype: "bfloat16" | "float32r" | "float32"
DT_NAME = os.environ.get("BASS_LINATTN_DT", "bfloat16")

_built = {}


def _np_dt(name):
    return {"bfloat16": ml_dtypes.bfloat16,
            "float32r": np.float32,
            "float32": np.float32}[name]


def _build(dt_name):
    import concourse.bass as bass
    import concourse.mybir as mybir
    from concourse import bacc
    from concourse.tile import TileContext

    DT = getattr(mybir.dt, dt_name)
    F32 = mybir.dt.float32
    AF = mybir.ActivationFunctionType
    ALU = mybir.AluOpType

    nc = bacc.Bacc("TRN2", target_bir_lowering=False, debug=False)

    # weights arrive pre-packed per 128-partition slab so every DMA slice
    # reads 1-2KB contiguous DRAM runs:
    #   wq/wk: [p, di, kt, c]   wv: [p, kt, c]   wo: [p, mt, n]
    xt = nc.dram_tensor("xt", (D, S), DT, kind="ExternalInput")
    wq = nc.dram_tensor("wq", (128, D * MC // 128), DT, kind="ExternalInput")
    wk = nc.dram_tensor("wk", (128, D * MC // 128), DT, kind="ExternalInput")
    wv = nc.dram_tensor("wv", (128, D * MC // 128), DT, kind="ExternalInput")
    wo = nc.dram_tensor("wo", (128, MC * D // 128), DT, kind="ExternalInput")
    bqkv = nc.dram_tensor("bqkv", (128, 12), F32, kind="ExternalInput")
    bvb = nc.dram_tensor("bvb", (128, MC), F32, kind="ExternalInput")
    msk = nc.dram_tensor("msk", (128, 256), F32, kind="ExternalInput")
    idn = nc.dram_tensor("idn", (128, 128), DT, kind="ExternalInput")
    out = nc.dram_tensor("out", (S, D), DT, kind="ExternalOutput")

    KT = D // 128          # 8 k-tiles of the contraction dim
    DT_TILES = MC // 128   # 4 tiles of per-core head dims

    with TileContext(nc) as tc:
        with (
            tc.tile_pool(name="consts", bufs=1) as consts,
            tc.tile_pool(name="xt_pool", bufs=2) as xt_pool,
            tc.tile_pool(name="qkvt", bufs=2) as qkvt,
            tc.tile_pool(name="nat", bufs=2) as nat,
            tc.tile_pool(name="attn_pool", bufs=8) as attn_pool,
            tc.tile_pool(name="attnT_pool", bufs=8) as attnT_pool,
            tc.tile_pool(name="state_pool", bufs=1) as state_pool,
            tc.tile_pool(name="small", bufs=8) as small,
            tc.tile_pool(name="evac", bufs=4) as evac,
            tc.tile_pool(name="psA", bufs=2, space="PSUM") as psA,
            tc.tile_pool(name="psB", bufs=2, space="PSUM") as psB,
        ):
            # ---- constants -------------------------------------------------
            # DMA order matters for startup latency: small consts first, then
            # weight slices in first-use order so the first qk_step can begin
            # after ~384KB instead of the full 5.3MB.
            # consts go out on the GpSimd engine's DMA ring so their many
            # small descriptors don't queue ahead of the first weight slices
            # on the Sync ring
            bias_sb = consts.tile([128, 12], F32)
            nc.gpsimd.dma_start(out=bias_sb, in_=bqkv[:, :])
            ident = consts.tile([128, 128], DT)
            nc.sync.dma_start(ident, idn[:, :])
            mask_sb = consts.tile([128, 256], F32)   # [triu | triu]
            nc.gpsimd.dma_start(out=mask_sb, in_=msk[:, :])
            bvb_sb = consts.tile([128, MC], F32)     # needed late (vnat)

            # HAM warmup: a burst of throwaway matmuls as soon as the first
            # const lands, so the PE's activity monitor un-throttles before
            # the real projections start
            warm_ps = psA.tile([128, 512], F32, tag="proj", bufs=2, name="warm")
            for _ in range(48):
                nc.tensor.matmul(warm_ps[0:64, 0:128], ident[:, 0:64],
                                 ident[:, :], start=True, stop=True)

            wq_sb = consts.tile([128, KT, MC], DT)
            wk_sb = consts.tile([128, KT, MC], DT)
            wv_sb = consts.tile([128, KT, MC], DT)
            wo_sb = consts.tile([128, DT_TILES, D], DT)

            def emit_w_slice(t, w, di):
                dsl = slice(di * 128, (di + 1) * 128)
                nc.sync.dma_start(
                    t[:, :, dsl],
                    w[:, di * KT * 128:(di + 1) * KT * 128]
                    .rearrange("p (kt c) -> p kt c", c=128),
                )
                if di == 0:
                    # paced HAM-keepalive matmul, gated on slice arrival
                    nc.tensor.matmul(warm_ps[0:128, 0:64], t[:, 0, dsl],
                                     ident[:, 0:64], start=True, stop=True)

            def emit_wv_wo():
                nc.gpsimd.dma_start(out=bvb_sb, in_=bvb[:, :])
                for k in range(KT):
                    nc.sync.dma_start(
                        wv_sb[:, k], wv[:, k * MC:(k + 1) * MC]
                    )
                for mt in range(DT_TILES):
                    nc.sync.dma_start(
                        wo_sb[:, mt], wo[:, mt * D:(mt + 1) * D]
                    )

            # ---- recurrent state [S|z], head pairs, block-diagonal --------
            state_b = state_pool.tile([128, HPC // 2, 130], DT)
            nc.vector.memset(state_b, 0.0)

            out_r = out.rearrange("(st p) n -> st p n", p=128)

            # per-block tiles, lazily created by the P stage
            T = {}

            def stage_p_steps(blk):
                """Projection-phase emission steps for block blk."""
                ssl = slice(blk * SB, (blk + 1) * SB)

                def dma_step():
                    xt_t = xt_pool.tile([128, KT, SB], DT, tag="xt",
                                        name=f"xt_{blk}")
                    T["xt", blk] = xt_t
                    # per-ktile DMAs: the k-accumulation loop of the first
                    # qk_step can start as soon as slice 0 lands
                    for k in range(KT):
                        nc.sync.dma_start(
                            xt_t[:, k],
                            xt.rearrange("(kt p) s -> p kt s", p=128)[:, k, ssl],
                        )
                    T["qt", blk] = qkvt.tile([128, DT_TILES, SB], DT, tag="qt",
                                             name=f"qt_{blk}")
                    T["kt", blk] = qkvt.tile([128, DT_TILES, SB], DT, tag="kt",
                                             name=f"kt_{blk}")
                    T["knat", blk] = nat.tile([128, NST, MC], DT, tag="knat",
                                              name=f"knat_{blk}")
                    vnat = nat.tile([128, NST, HPC * 65], DT, tag="vnat",
                                    name=f"vnat_{blk}")
                    T["vnat", blk] = vnat
                    nc.vector.memset(
                        vnat.rearrange("p st (h e) -> p st h e", e=65)
                            [:, :, :, 64:65], 1.0
                    )
                yield dma_step

                for di in range(DT_TILES):
                    def qk_step(di=di):
                        dsl = slice(di * 128, (di + 1) * 128)
                        xt_t = T["xt", blk]
                        # interleave the Q and K accumulation groups per
                        # k-slice: each arriving xt slice feeds two matmuls,
                        # so the PE no longer outpaces the x DMA on block 0,
                        # and group boundaries stagger
                        pss = {}
                        for pname in ("q", "k"):
                            pss[pname] = psA.tile(
                                [128, SB], F32, tag="proj", bufs=2,
                                name=f"ps_{pname}{di}_{blk}")
                        for k in range(KT):
                            for pname, w_sb in (("q", wq_sb), ("k", wk_sb)):
                                nc.tensor.matmul(
                                    pss[pname], w_sb[:, k, dsl], xt_t[:, k],
                                    start=(k == 0), stop=(k == KT - 1),
                                )
                        for pname, bcol, dkey in (
                            ("q", di, "qt"),
                            ("k", 4 + di, "kt"),
                        ):
                            dst = T[dkey, blk]
                            ps = pss[pname]
                            bias_ap = bias_sb[:, bcol:bcol + 1]
                            # elu(u)+1 = min(exp(u),1) + max(u,0), u = ps+bias
                            e = evac.tile([128, SB], DT, tag="e",
                                          name=f"e_{pname}{di}_{blk}")
                            nc.scalar.activation(e, ps, AF.Exp, bias=bias_ap)
                            r = evac.tile([128, SB], DT, tag="r",
                                          name=f"r_{pname}{di}_{blk}")
                            nc.scalar.activation(r, ps, AF.Relu, bias=bias_ap)
                            nc.vector.scalar_tensor_tensor(
                                out=dst[:, di], in0=e, scalar=1.0, in1=r,
                                op0=ALU.min, op1=ALU.add,
                            )
                        if dkey == "kt":
                            pass
                    yield qk_step

                # natural-layout K via PE transpose of elu'd Kt
                for di in range(DT_TILES):
                    def ktr_step(di=di):
                        kt_t = T["kt", blk]
                        knat = T["knat", blk]
                        for st in range(NST):
                            csl = slice(st * 128, (st + 1) * 128)
                            tr = psB.tile([128, 128], DT, tag="at", bufs=3,
                                          name=f"trk{di}_{st}_{blk}")
                            nc.tensor.transpose(tr, kt_t[:, di, csl], ident)
                            nc.any.tensor_copy(
                                knat[:, st, di * 128:(di + 1) * 128], tr
                            )
                    yield ktr_step

                # natural-layout V via direct (natural-out) projection
                for st in range(NST):
                    def vnat_step(st=st):
                        xt_t = T["xt", blk]
                        vnat = T["vnat", blk]
                        stsl = slice(st * 128, (st + 1) * 128)
                        ps = psA.tile([128, MC], F32, tag="proj", bufs=2,
                                      name=f"ps_vn{st}_{blk}")
                        for k in range(KT):
                            nc.tensor.matmul(
                                ps, T["xt", blk][:, k, stsl], wv_sb[:, k],
                                start=(k == 0), stop=(k == KT - 1),
                            )
                        nc.vector.tensor_add(
                            vnat.rearrange("p st (h e) -> p st h e", e=65)
                                [:, st, :, 0:64],
                            ps.rearrange("p (h e) -> p h e", e=64),
                            bvb_sb.rearrange("p (h e) -> p h e", e=64),
                        )
                    yield vnat_step

            def make_oproj(blk, st, nb):
                def oproj_step(blk=blk, st=st, nb=nb):
                    csl = slice(st * 128, (st + 1) * 128)
                    nsl = slice(nb * 512, (nb + 1) * 512)
                    ops = psA.tile([128, 512], F32, tag="proj", bufs=2,
                                   name=f"ops{st}_{nb}_{blk}")
                    for p in range(DT_TILES):
                        nc.tensor.matmul(
                            ops, T["attnT", blk][p][:, csl],
                            wo_sb[:, p, nsl],
                            start=(p == 0), stop=(p == DT_TILES - 1),
                        )
                    ob = evac.tile([128, 512], DT, tag="ob",
                                   name=f"ob{st}_{nb}_{blk}")
                    nc.scalar.copy(ob, ops)
                    nc.sync.dma_start(out_r[blk * NST + st, :, nsl], ob)
                return oproj_step

            def stage_a_steps(blk):
                """Attention + output-projection emission steps for block blk."""
                def alloc_step():
                    T["attn", blk] = [
                        attn_pool.tile([128, MC], DT, tag="attn",
                                       name=f"attn{st}_{blk}")
                        for st in range(NST)
                    ]
                    T["attnT", blk] = [
                        attnT_pool.tile([128, SB], DT, tag="attnT",
                                        name=f"attnT{p}_{blk}")
                        for p in range(DT_TILES)
                    ]
                yield alloc_step

                # masked AT for every (chunk, pair) has no state dependency:
                # hoist it all to the front of the block so the PE runs it
                # dense (and warm) before the serial state-chain phase
                atm = {}
                for cc in range(NST):
                    for hp in range(HPC // 2):
                        def at_step(cc=cc, hp=hp):
                            csl = slice(cc * 128, (cc + 1) * 128)
                            qt_t, kt_t = T["qt", blk], T["kt", blk]
                            # per-head AT psum tiles: the two matmuls run
                            # concurrently (different PE row groups), so they
                            # must write DIFFERENT psum banks
                            at_m = small.tile([128, 256], DT, tag="atm",
                                              bufs=20,
                                              name=f"atm{hp}_{cc}_{blk}")
                            atm[cc, hp] = at_m
                            for o in range(2):
                                pr = slice(o * 64, o * 64 + 64)
                                at_ps = psB.tile([128, 128], F32, tag="at", bufs=3,
                                                 name=f"at{hp}{o}_{cc}_{blk}")
                                nc.tensor.matmul(
                                    at_ps, kt_t[pr, hp, csl], qt_t[pr, hp, csl],
                                    start=True, stop=True,
                                )
                                nc.vector.tensor_mul(
                                    at_m[:, o * 128:(o + 1) * 128], at_ps,
                                    mask_sb[:, 0:128],
                                )
                        yield at_step

                for cc in range(NST):
                    for hp in range(HPC // 2):
                        if blk == NBLK - 1 and cc > 0 and hp in (1, 2):
                            # spread the previous chunk's HAM-visible oproj
                            # matmuls through the chain's stall-prone stretch
                            yield make_oproj(blk, cc - 1, hp - 1)

                        def pair_step(cc=cc, hp=hp):
                            csl = slice(cc * 128, (cc + 1) * 128)
                            qt_t = T["qt", blk]
                            knat, vnat = T["knat", blk], T["vnat", blk]
                            at_m = atm[cc, hp]
                            # acc = [num_e | den_e | num_o | den_o]
                            # NOTE: the inter matmul opens the accumulation
                            # group (start=True zeroes the whole PSUM bank;
                            # sub-bank disjoint start=True writes would
                            # clobber each other).
                            acc = psB.tile([128, 130], F32, tag="acc", bufs=2,
                                           name=f"acc{hp}_{cc}_{blk}")
                            nc.tensor.matmul(
                                acc, qt_t[:, hp, csl], state_b[:, hp],
                                start=True, stop=False, skip_group_check=True,
                            )
                            for o in range(2):
                                h = 2 * hp + o
                                nc.tensor.matmul(
                                    acc[:, o * 65:o * 65 + 65],
                                    at_m[:, o * 128:(o + 1) * 128],
                                    vnat[:, cc, h * 65:(h + 1) * 65],
                                    start=False, stop=(o == 1),
                                    skip_group_check=True,
                                )
                            # state += K_c^T [V|1] (pair; off-diag blocks unused)
                            stp = psB.tile([128, 130], F32, tag="state", bufs=1,
                                           name=f"stp{hp}_{cc}_{blk}")
                            nc.tensor.matmul(
                                stp, knat[:, cc, hp * 128:(hp + 1) * 128],
                                vnat[:, cc, hp * 130:(hp + 1) * 130],
                                start=True, stop=True,
                            )
                            # paired reciprocal of the two den columns
                            rec = small.tile([128, 2], F32, tag="rec",
                                             name=f"rec{hp}_{cc}_{blk}")
                            nc.vector.reciprocal(rec, acc[:, 64:130:65])
                            for o in range(2):
                                pr = slice(o * 64, o * 64 + 64)
                                osl = slice(o * 65, o * 65 + 65)
                                nc.vector.tensor_add(
                                    state_b[pr, hp, osl], state_b[pr, hp, osl],
                                    stp[pr, osl],
                                )
                                h = 2 * hp + o
                                nc.vector.tensor_scalar_mul(
                                    T["attn", blk][cc][:, h * 64:(h + 1) * 64],
                                    acc[:, o * 65:o * 65 + 64], rec[:, o:o + 1],
                                )
                        yield pair_step

                    def attnT_step(cc=cc):
                        csl = slice(cc * 128, (cc + 1) * 128)
                        for p in range(DT_TILES):
                            trA = psB.tile([128, 128], DT, tag="at", bufs=3,
                                           name=f"trA{p}_{cc}_{blk}")
                            nc.tensor.transpose(
                                trA, T["attn", blk][cc][:, p * 128:(p + 1) * 128],
                                ident,
                            )
                            nc.any.tensor_copy(T["attnT", blk][p][:, csl], trA)
                    yield attnT_step

                    if blk == NBLK - 1 and cc == NST - 1:
                        # the final chunk's oproj has no later chain to hide in
                        for nb in range(D // 512):
                            yield make_oproj(blk, cc, nb)

                if blk < NBLK - 1:
                    for st in range(NST):
                        for nb in range(D // 512):
                            yield make_oproj(blk, st, nb)

            # ---- software-pipelined emission ------------------------------
            # DMAs drain roughly in emission order: get block 0's x and the
            # first weight slices in front of the bulk weights so the PE can
            # start at ~10us instead of ~27us.
            emit_w_slice(wq_sb, wq, 0)
            p0_steps = list(stage_p_steps(0))
            p0_steps[0]()            # xt block-0 DMAs + vnat init
            emit_w_slice(wk_sb, wk, 0)
            for di in range(1, DT_TILES):
                emit_w_slice(wq_sb, wq, di)
                emit_w_slice(wk_sb, wk, di)
            emit_wv_wo()
            for step in p0_steps[1:]:
                step()
            for blk in range(NBLK):
                a_steps = list(stage_a_steps(blk))
                p_steps = list(stage_p_steps(blk + 1)) if blk + 1 < NBLK else []
                # interleave: spread p_steps evenly through a_steps
                na, npp = len(a_steps), len(p_steps)
                pi = 0
                for i, astep in enumerate(a_steps):
                    astep()
                    while pi < npp and (i + 1) * npp >= (pi + 1) * na:
                        p_steps[pi]()
                        pi += 1
                while pi < npp:
                    p_steps[pi]()
                    pi += 1

    nc.compile()
    return nc


def _prep_inputs(x, Wq, bq, Wk, bk, Wv, bv, Wo, bo, np_dt):
    f32 = np.float32
    tri = np.triu(np.ones((128, 128), f32))  # mask[j,i] = 1 iff j <= i
    mask_tri = np.concatenate([tri, tri], axis=1)  # paired heads
    ident = np.eye(128, dtype=np_dt)
    in_maps = []
    for c in range(8):
        b, hh = divmod(c, 2)
        cols = slice(hh * MC, (hh + 1) * MC)
        bqkv = np.concatenate(
            [np.asarray(v[cols], f32).reshape(4, 128).T for v in (bq, bk, bv)],
            axis=1,
        ).astype(f32)
        # pack weights so each kernel DMA slice is a long contiguous DRAM run:
        #   wq/wk [p, di, kt, c]; wv [p, kt, c]; wo [p, mt, n]
        wq_p = (np.asarray(Wq, f32)[:, cols].reshape(8, 128, 4, 128)
                .transpose(1, 2, 0, 3).reshape(128, 4096))
        wk_p = (np.asarray(Wk, f32)[:, cols].reshape(8, 128, 4, 128)
                .transpose(1, 2, 0, 3).reshape(128, 4096))
        wv_p = (np.asarray(Wv, f32)[:, cols].reshape(8, 128, 512)
                .transpose(1, 0, 2).reshape(128, 4096))
        wo_p = (np.asarray(Wo, f32)[cols, :].reshape(4, 128, 1024)
                .transpose(1, 0, 2).reshape(128, 4096))
        in_maps.append({
            "xt": np.ascontiguousarray(np.asarray(x[b], f32).T).astype(np_dt),
            "wq": np.ascontiguousarray(wq_p).astype(np_dt),
            "wk": np.ascontiguousarray(wk_p).astype(np_dt),
            "wv": np.ascontiguousarray(wv_p).astype(np_dt),
            "wo": np.ascontiguousarray(wo_p).astype(np_dt),
            "bqkv": np.ascontiguousarray(bqkv),
            "bvb": np.ascontiguousarray(
                np.tile(np.asarray(bv, f32)[cols][None, :], (128, 1))
            ),
            "msk": mask_tri,
            "idn": ident,
        })
    return in_maps


def run(inputs, trace=False):
    """Run the kernel; returns (full_output, BassKernelResults)."""
    from concourse.bass_utils import run_bass_kernel_spmd

    dt_name = DT_NAME
    if dt_name not in _built:
        _built[dt_name] = _build(dt_name)
    nc = _built[dt_name]

    x = np.asarray(inputs["x"], np.float32)
    bo = np.asarray(inputs["bo"], np.float32)
    in_maps = _prep_inputs(
        x, inputs["Wq"], inputs["bq"], inputs["Wk"], inputs["bk"],
        inputs["Wv"], inputs["bv"], inputs["Wo"], bo, _np_dt(dt_name),
    )
    res = run_bass_kernel_spmd(
        nc, in_maps, core_ids=list(range(8)), trace=trace,
        trace_cores=list(range(8)) if trace else None,
    )
    outs = [np.asarray(r["out"], np.float32) for r in res.results]
    full = np.empty((B, S, D), np.float32)
    for b in range(B):
        full[b] = outs[2 * b] + outs[2 * b + 1] + bo[None, :]
    return full, res


def kernel(**inputs):
    full, _ = run(inputs, trace=False)
    return full



# revision 74
# speedup vs baseline: 1.0513x; 1.0513x over previous
"""Trainium2 Bass kernel for MultiHeadLinearAttention.

Problem: B=4, S=2048, D=1024, H=16 heads of hd=64.
  q,k,v = x@W + b ; q,k = elu(q|k)+1
  recurrent scan: s += k_t v_t^T ; z += k_t ; out_t = (q_t s)/(q_t z + 1e-6)
  y = out @ Wo + bo

Strategy (8 NeuronCores):
  core c -> batch b = c//2, heads hh = (c%2)*8 .. +8  (column-sliced Wq/Wk/Wv,
  row-sliced Wo; the two cores of a batch produce partial output-projection
  sums that the host adds together, plus bo).

  Linear attention is computed in chunked (block-parallel) form with chunk
  C=128: per chunk and head,
    AT[j,i] = sum_d k[j,d] q[i,d]          (j,i in chunk; masked to j<=i)
    acc[i,0:64]  = sum_j AT_m[j,i] v[j,:] + q_i @ S_pre
    acc[i,64]    = sum_j AT_m[j,i] * 1    + q_i @ z_pre    ([V|1] interleave)
    out_i = acc[i,0:64] / (acc[i,64] + 1e-6)
    [S|z] += K_c^T [V_c|1]
  elu(x)+1 == min(exp(x),1) + max(x,0).

  The host feeds x[b].T so projections contract along the partition dim and
  produce Q^T,K^T,V^T directly; K/V natural layouts come from PE transposes.
"""

import os
import sys

import numpy as np
import ml_dtypes

sys.path.insert(0, "/opt/trn_rl_repo")

B, S, D = 4, 2048, 1024
H, HD = 16, 64
HPC = 8           # heads per core
MC = HPC * HD     # 512 head-dims per core
C = 128           # attention chunk
SB = 512          # s-block
NBLK = S // SB    # 4
NST = SB // C     # s-tiles (=chunks) per block

# matmul operand dtype: "bfloat16" | "float32r" | "float32"
DT_NAME = os.environ.get("BASS_LINATTN_DT", "bfloat16")

_built = {}


def _np_dt(name):
    return {"bfloat16": ml_dtypes.bfloat16,
            "float32r": np.float32,
            "float32": np.float32}[name]


def _build(dt_name):
    import concourse.bass as bass
    import concourse.mybir as mybir
    from concourse import bacc
    from concourse.tile import TileContext

    DT = getattr(mybir.dt, dt_name)
    F32 = mybir.dt.float32
    AF = mybir.ActivationFunctionType
    ALU = mybir.AluOpType

    nc = bacc.Bacc("TRN2", target_bir_lowering=False, debug=False)

    # weights arrive pre-packed per 128-partition slab so every DMA slice
    # reads 1-2KB contiguous DRAM runs:
    #   wq/wk: [p, di, kt, c]   wv: [p, kt, c]   wo: [p, mt, n]
    xt = nc.dram_tensor("xt", (D, S), DT, kind="ExternalInput")
    wq = nc.dram_tensor("wq", (128, D * MC // 128), DT, kind="ExternalInput")
    wk = nc.dram_tensor("wk", (128, D * MC // 128), DT, kind="ExternalInput")
    wv = nc.dram_tensor("wv", (128, D * MC // 128), DT, kind="ExternalInput")
    wo = nc.dram_tensor("wo", (128, MC * D // 128), DT, kind="ExternalInput")
    bqkv = nc.dram_tensor("bqkv", (128, 12), F32, kind="ExternalInput")
    bvb = nc.dram_tensor("bvb", (128, MC), F32, kind="ExternalInput")
    msk = nc.dram_tensor("msk", (128, 256), F32, kind="ExternalInput")
    idn = nc.dram_tensor("idn", (128, 128), DT, kind="ExternalInput")
    out = nc.dram_tensor("out", (S, D), DT, kind="ExternalOutput")

    KT = D // 128          # 8 k-tiles of the contraction dim
    DT_TILES = MC // 128   # 4 tiles of per-core head dims

    with TileContext(nc) as tc:
        with (
            tc.tile_pool(name="consts", bufs=1) as consts,
            tc.tile_pool(name="xt_pool", bufs=2) as xt_pool,
            tc.tile_pool(name="qkvt", bufs=2) as qkvt,
            tc.tile_pool(name="nat", bufs=2) as nat,
            tc.tile_pool(name="attn_pool", bufs=8) as attn_pool,
            tc.tile_pool(name="attnT_pool", bufs=8) as attnT_pool,
            tc.tile_pool(name="state_pool", bufs=1) as state_pool,
            tc.tile_pool(name="small", bufs=8) as small,
            tc.tile_pool(name="evac", bufs=4) as evac,
            tc.tile_pool(name="psA", bufs=2, space="PSUM") as psA,
            tc.tile_pool(name="psB", bufs=2, space="PSUM") as psB,
        ):
            # ---- constants -------------------------------------------------
            # DMA order matters for startup latency: small consts first, then
            # weight slices in first-use order so the first qk_step can begin
            # after ~384KB instead of the full 5.3MB.
            # consts go out on the GpSimd engine's DMA ring so their many
            # small descriptors don't queue ahead of the first weight slices
            # on the Sync ring
            bias_sb = consts.tile([128, 12], F32)
            nc.gpsimd.dma_start(out=bias_sb, in_=bqkv[:, :])
            ident = consts.tile([128, 128], DT)
            nc.sync.dma_start(ident, idn[:, :])
            mask_sb = consts.tile([128, 256], F32)   # [triu | triu]
            nc.gpsimd.dma_start(out=mask_sb, in_=msk[:, :])
            bvb_sb = consts.tile([128, MC], F32)     # needed late (vnat)

            # HAM warmup: a burst of throwaway matmuls as soon as the first
            # const lands, so the PE's activity monitor un-throttles before
            # the real projections start
            warm_ps = psA.tile([128, 512], F32, tag="proj", bufs=2, name="warm")
            for _ in range(48):
                nc.tensor.matmul(warm_ps[0:64, 0:128], ident[:, 0:64],
                                 ident[:, :], start=True, stop=True)

            wq_sb = consts.tile([128, KT, MC], DT)
            wk_sb = consts.tile([128, KT, MC], DT)
            wv_sb = consts.tile([128, KT, MC], DT)
            wo_sb = consts.tile([128, DT_TILES, D], DT)

            def emit_w_slice(t, w, di):
                dsl = slice(di * 128, (di + 1) * 128)
                nc.sync.dma_start(
                    t[:, :, dsl],
                    w[:, di * KT * 128:(di + 1) * KT * 128]
                    .rearrange("p (kt c) -> p kt c", c=128),
                )
                if di == 0:
                    # paced HAM-keepalive matmul, gated on slice arrival
                    nc.tensor.matmul(warm_ps[0:128, 0:64], t[:, 0, dsl],
                                     ident[:, 0:64], start=True, stop=True)

            def emit_wv_wo():
                nc.gpsimd.dma_start(out=bvb_sb, in_=bvb[:, :])
                for k in range(KT):
                    nc.sync.dma_start(
                        wv_sb[:, k], wv[:, k * MC:(k + 1) * MC]
                    )
                for mt in range(DT_TILES):
                    nc.sync.dma_start(
                        wo_sb[:, mt], wo[:, mt * D:(mt + 1) * D]
                    )

            # ---- recurrent state [S|z], head pairs, block-diagonal --------
            state_b = state_pool.tile([128, HPC // 2, 130], DT)
            nc.vector.memset(state_b, 0.0)

            out_r = out.rearrange("(st p) n -> st p n", p=128)

            # per-block tiles, lazily created by the P stage
            T = {}

            def stage_p_steps(blk):
                """Projection-phase emission steps for block blk."""
                ssl = slice(blk * SB, (blk + 1) * SB)

                def dma_step():
                    xt_t = xt_pool.tile([128, KT, SB], DT, tag="xt",
                                        name=f"xt_{blk}")
                    T["xt", blk] = xt_t
                    # per-ktile DMAs: the k-accumulation loop of the first
                    # qk_step can start as soon as slice 0 lands
                    for k in range(KT):
                        nc.sync.dma_start(
                            xt_t[:, k],
                            xt.rearrange("(kt p) s -> p kt s", p=128)[:, k, ssl],
                        )
                    T["qt", blk] = qkvt.tile([128, DT_TILES, SB], DT, tag="qt",
                                             name=f"qt_{blk}")
                    T["kt", blk] = qkvt.tile([128, DT_TILES, SB], DT, tag="kt",
                                             name=f"kt_{blk}")
                    T["knat", blk] = nat.tile([128, NST, MC], DT, tag="knat",
                                              name=f"knat_{blk}")
                    vnat = nat.tile([128, NST, HPC * 65], DT, tag="vnat",
                                    name=f"vnat_{blk}")
                    T["vnat", blk] = vnat
                    nc.vector.memset(
                        vnat.rearrange("p st (h e) -> p st h e", e=65)
                            [:, :, :, 64:65], 1.0
                    )
                yield dma_step

                for di in range(DT_TILES):
                    def qk_step(di=di):
                        dsl = slice(di * 128, (di + 1) * 128)
                        xt_t = T["xt", blk]
                        # interleave the Q and K accumulation groups per
                        # k-slice: each arriving xt slice feeds two matmuls,
                        # so the PE no longer outpaces the x DMA on block 0,
                        # and group boundaries stagger
                        pss = {}
                        for pname in ("q", "k"):
                            pss[pname] = psA.tile(
                                [128, SB], F32, tag="proj", bufs=2,
                                name=f"ps_{pname}{di}_{blk}")
                        for k in range(KT):
                            for pname, w_sb in (("q", wq_sb), ("k", wk_sb)):
                                nc.tensor.matmul(
                                    pss[pname], w_sb[:, k, dsl], xt_t[:, k],
                                    start=(k == 0), stop=(k == KT - 1),
                                )
                        for pname, bcol, dkey in (
                            ("q", di, "qt"),
                            ("k", 4 + di, "kt"),
                        ):
                            dst = T[dkey, blk]
                            ps = pss[pname]
                            bias_ap = bias_sb[:, bcol:bcol + 1]
                            # elu(u)+1 = min(exp(u),1) + max(u,0), u = ps+bias
                            e = evac.tile([128, SB], DT, tag="e",
                                          name=f"e_{pname}{di}_{blk}")
                            nc.scalar.activation(e, ps, AF.Exp, bias=bias_ap)
                            r = evac.tile([128, SB], DT, tag="r",
                                          name=f"r_{pname}{di}_{blk}")
                            nc.scalar.activation(r, ps, AF.Relu, bias=bias_ap)
                            nc.vector.scalar_tensor_tensor(
                                out=dst[:, di], in0=e, scalar=1.0, in1=r,
                                op0=ALU.min, op1=ALU.add,
                            )
                        if dkey == "kt":
                            pass
                    yield qk_step

                # natural-layout K via PE transpose of elu'd Kt
                for di in range(DT_TILES):
                    def ktr_step(di=di):
                        kt_t = T["kt", blk]
                        knat = T["knat", blk]
                        for st in range(NST):
                            csl = slice(st * 128, (st + 1) * 128)
                            tr = psB.tile([128, 128], DT, tag="at", bufs=3,
                                          name=f"trk{di}_{st}_{blk}")
                            nc.tensor.transpose(tr, kt_t[:, di, csl], ident)
                            nc.any.tensor_copy(
                                knat[:, st, di * 128:(di + 1) * 128], tr
                            )
                    yield ktr_step

                # natural-layout V via direct (natural-out) projection
                for st in range(NST):
                    def vnat_step(st=st):
                        xt_t = T["xt", blk]
                        vnat = T["vnat", blk]
                        stsl = slice(st * 128, (st + 1) * 128)
                        ps = psA.tile([128, MC], F32, tag="proj", bufs=2,
                                      name=f"ps_vn{st}_{blk}")
                        for k in range(KT):
                            nc.tensor.matmul(
                                ps, T["xt", blk][:, k, stsl], wv_sb[:, k],
                                start=(k == 0), stop=(k == KT - 1),
                            )
                        nc.vector.tensor_add(
                            vnat.rearrange("p st (h e) -> p st h e", e=65)
                                [:, st, :, 0:64],
                            ps.rearrange("p (h e) -> p h e", e=64),
                            bvb_sb.rearrange("p (h e) -> p h e", e=64),
                        )
                    yield vnat_step

            def make_oproj(blk, st, nb):
                def oproj_step(blk=blk, st=st, nb=nb):
                    csl = slice(st * 128, (st + 1) * 128)
                    nsl = slice(nb * 512, (nb + 1) * 512)
                    ops = psA.tile([128, 512], F32, tag="proj", bufs=2,
                                   name=f"ops{st}_{nb}_{blk}")
                    for p in range(DT_TILES):
                        nc.tensor.matmul(
                            ops, T["attnT", blk][p][:, csl],
                            wo_sb[:, p, nsl],
                            start=(p == 0), stop=(p == DT_TILES - 1),
                        )
                    ob = evac.tile([128, 512], DT, tag="ob",
                                   name=f"ob{st}_{nb}_{blk}")
                    nc.scalar.copy(ob, ops)
                    nc.sync.dma_start(out_r[blk * NST + st, :, nsl], ob)
                return oproj_step

            def stage_a_steps(blk):
                """Attention + output-projection emission steps for block blk."""
                def alloc_step():
                    T["attn", blk] = [
                        attn_pool.tile([128, MC], DT, tag="attn",
                                       name=f"attn{st}_{blk}")
                        for st in range(NST)
                    ]
                    T["attnT", blk] = [
                        attnT_pool.tile([128, SB], DT, tag="attnT",
                                        name=f"attnT{p}_{blk}")
                        for p in range(DT_TILES)
                    ]
                yield alloc_step

                # masked AT for every (chunk, pair) has no state dependency:
                # hoist it all to the front of the block so the PE runs it
                # dense (and warm) before the serial state-chain phase
                atm = {}
                for cc in range(NST):
                    for hp in range(HPC // 2):
                        def at_step(cc=cc, hp=hp):
                            csl = slice(cc * 128, (cc + 1) * 128)
                            qt_t, kt_t = T["qt", blk], T["kt", blk]
                            # per-head AT psum tiles: the two matmuls run
                            # concurrently (different PE row groups), so they
                            # must write DIFFERENT psum banks
                            at_m = small.tile([128, 256], DT, tag="atm",
                                              bufs=20,
                                              name=f"atm{hp}_{cc}_{blk}")
                            atm[cc, hp] = at_m
                            for o in range(2):
                                pr = slice(o * 64, o * 64 + 64)
                                at_ps = psB.tile([128, 128], F32, tag="at", bufs=3,
                                                 name=f"at{hp}{o}_{cc}_{blk}")
                                nc.tensor.matmul(
                                    at_ps, kt_t[pr, hp, csl], qt_t[pr, hp, csl],
                                    start=True, stop=True,
                                )
                                nc.vector.tensor_mul(
                                    at_m[:, o * 128:(o + 1) * 128], at_ps,
                                    mask_sb[:, 0:128],
                                )
                        yield at_step

                for cc in range(NST):
                    for hp in range(HPC // 2):
                        def pair_step(cc=cc, hp=hp):
                            csl = slice(cc * 128, (cc + 1) * 128)
                            qt_t = T["qt", blk]
                            knat, vnat = T["knat", blk], T["vnat", blk]
                            at_m = atm[cc, hp]
                            # acc = [num_e | den_e | num_o | den_o]
                            # NOTE: the inter matmul opens the accumulation
                            # group (start=True zeroes the whole PSUM bank;
                            # sub-bank disjoint start=True writes would
                            # clobber each other).
                            acc = psB.tile([128, 130], F32, tag="acc", bufs=2,
                                           name=f"acc{hp}_{cc}_{blk}")
                            nc.tensor.matmul(
                                acc, qt_t[:, hp, csl], state_b[:, hp],
                                start=True, stop=False, skip_group_check=True,
                            )
                            for o in range(2):
                                h = 2 * hp + o
                                nc.tensor.matmul(
                                    acc[:, o * 65:o * 65 + 65],
                                    at_m[:, o * 128:(o + 1) * 128],
                                    vnat[:, cc, h * 65:(h + 1) * 65],
                                    start=False, stop=(o == 1),
                                    skip_group_check=True,
                                )
                            # state += K_c^T [V|1] (pair; off-diag blocks unused)
                            stp = psB.tile([128, 130], F32, tag="state", bufs=1,
                                           name=f"stp{hp}_{cc}_{blk}")
                            nc.tensor.matmul(
                                stp, knat[:, cc, hp * 128:(hp + 1) * 128],
                                vnat[:, cc, hp * 130:(hp + 1) * 130],
                                start=True, stop=True,
                            )
                            # paired reciprocal of the two den columns
                            rec = small.tile([128, 2], F32, tag="rec",
                                             name=f"rec{hp}_{cc}_{blk}")
                            nc.vector.reciprocal(rec, acc[:, 64:130:65])
                            for o in range(2):
                                pr = slice(o * 64, o * 64 + 64)
                                osl = slice(o * 65, o * 65 + 65)
                                nc.vector.tensor_add(
                                    state_b[pr, hp, osl], state_b[pr, hp, osl],
                                    stp[pr, osl],
                                )
                                h = 2 * hp + o
                                nc.vector.tensor_scalar_mul(
                                    T["attn", blk][cc][:, h * 64:(h + 1) * 64],
                                    acc[:, o * 65:o * 65 + 64], rec[:, o:o + 1],
                                )
                        yield pair_step

                    def attnT_step(cc=cc):
                        csl = slice(cc * 128, (cc + 1) * 128)
                        for p in range(DT_TILES):
                            trA = psB.tile([128, 128], DT, tag="at", bufs=3,
                                           name=f"trA{p}_{cc}_{blk}")
                            nc.tensor.transpose(
                                trA, T["attn", blk][cc][:, p * 128:(p + 1) * 128],
                                ident,
                            )
                            nc.any.tensor_copy(T["attnT", blk][p][:, csl], trA)
                    yield attnT_step

                    if blk == NBLK - 1:
                        # last block has no next-block projections to overlap
                        # with: emit its oproj per-chunk to keep the PE dense
                        for nb in range(D // 512):
                            yield make_oproj(blk, cc, nb)

                if blk < NBLK - 1:
                    for st in range(NST):
                        for nb in range(D // 512):
                            yield make_oproj(blk, st, nb)

            # ---- software-pipelined emission ------------------------------
            # DMAs drain roughly in emission order: get block 0's x and the
            # first weight slices in front of the bulk weights so the PE can
            # start at ~10us instead of ~27us.
            emit_w_slice(wq_sb, wq, 0)
            p0_steps = list(stage_p_steps(0))
            p0_steps[0]()            # xt block-0 DMAs + vnat init
            emit_w_slice(wk_sb, wk, 0)
            for di in range(1, DT_TILES):
                emit_w_slice(wq_sb, wq, di)
                emit_w_slice(wk_sb, wk, di)
            emit_wv_wo()
            for step in p0_steps[1:]:
                step()
            for blk in range(NBLK):
                a_steps = list(stage_a_steps(blk))
                p_steps = list(stage_p_steps(blk + 1)) if blk + 1 < NBLK else []
                # interleave: spread p_steps evenly through a_steps
                na, npp = len(a_steps), len(p_steps)
                pi = 0
                for i, astep in enumerate(a_steps):
                    astep()
                    while pi < npp and (i + 1) * npp >= (pi + 1) * na:
                        p_steps[pi]()
                        pi += 1
                while pi < npp:
                    p_steps[pi]()
                    pi += 1

    nc.compile()
    return nc


def _prep_inputs(x, Wq, bq, Wk, bk, Wv, bv, Wo, bo, np_dt):
    f32 = np.float32
    tri = np.triu(np.ones((128, 128), f32))  # mask[j,i] = 1 iff j <= i
    mask_tri = np.concatenate([tri, tri], axis=1)  # paired heads
    ident = np.eye(128, dtype=np_dt)
    in_maps = []
    for c in range(8):
        b, hh = divmod(c, 2)
        cols = slice(hh * MC, (hh + 1) * MC)
        bqkv = np.concatenate(
            [np.asarray(v[cols], f32).reshape(4, 128).T for v in (bq, bk, bv)],
            axis=1,
        ).astype(f32)
        # pack weights so each kernel DMA slice is a long contiguous DRAM run:
        #   wq/wk [p, di, kt, c]; wv [p, kt, c]; wo [p, mt, n]
        wq_p = (np.asarray(Wq, f32)[:, cols].reshape(8, 128, 4, 128)
                .transpose(1, 2, 0, 3).reshape(128, 4096))
        wk_p = (np.asarray(Wk, f32)[:, cols].reshape(8, 128, 4, 128)
                .transpose(1, 2, 0, 3).reshape(128, 4096))
        wv_p = (np.asarray(Wv, f32)[:, cols].reshape(8, 128, 512)
                .transpose(1, 0, 2).reshape(128, 4096))
        wo_p = (np.asarray(Wo, f32)[cols, :].reshape(4, 128, 1024)
                .transpose(1, 0, 2).reshape(128, 4096))
        in_maps.append({
            "xt": np.ascontiguousarray(np.asarray(x[b], f32).T).astype(np_dt),
            "wq": np.ascontiguousarray(wq_p).astype(np_dt),
            "wk": np.ascontiguousarray(wk_p).astype(np_dt),
            "wv": np.ascontiguousarray(wv_p).astype(np_dt),
            "wo": np.ascontiguousarray(wo_p).astype(np_dt),
            "bqkv": np.ascontiguousarray(bqkv),
            "bvb": np.ascontiguousarray(
                np.tile(np.asarray(bv, f32)[cols][None, :], (128, 1))
            ),
            "msk": mask_tri,
            "idn": ident,
        })
    return in_maps


def run(inputs, trace=False):
    """Run the kernel; returns (full_output, BassKernelResults)."""
    from concourse.bass_utils import run_bass_kernel_spmd

    dt_name = DT_NAME
    if dt_name not in _built:
        _built[dt_name] = _build(dt_name)
    nc = _built[dt_name]

    x = np.asarray(inputs["x"], np.float32)
    bo = np.asarray(inputs["bo"], np.float32)
    in_maps = _prep_inputs(
        x, inputs["Wq"], inputs["bq"], inputs["Wk"], inputs["bk"],
        inputs["Wv"], inputs["bv"], inputs["Wo"], bo, _np_dt(dt_name),
    )
    res = run_bass_kernel_spmd(
        nc, in_maps, core_ids=list(range(8)), trace=trace,
        trace_cores=list(range(8)) if trace else None,
    )
    outs = [np.asarray(r["out"], np.float32) for r in res.results]
    full = np.empty((B, S, D), np.float32)
    for b in range(B):
        full[b] = outs[2 * b] + outs[2 * b + 1] + bo[None, :]
    return full, res


def kernel(**inputs):
    full, _ = run(inputs, trace=False)
    return full

